# revision 61
# baseline (speedup 1.0000x reference)
"""Grouped Query Attention on 8 TRN2 NeuronCores (~404us HW, vs 724us
f32 baseline).

Sharding: batch x s_q-quarter (core c -> batch c//4, query rows
[512*(c%4), 512*(c%4+1))). Each core holds only its own s-quarter of
x^T, computes Q^T for its 512 query rows plus K^T/V for that quarter,
AllGathers K/V across the 4 cores of its batch (overlapped with the Q
projection), runs attention for all 16 heads over its query rows, and
writes a disjoint [512, 2048] out-projection slice. Unsharding is
concatenation.

Design notes (each validated against a neuron-profile trace):
- bf16 inputs/weights (host-cast): halves HBM traffic, same PE rate as
  f32r; rel-err ~5e-3 vs the 2e-2 gate.
- All intermediates (Q^T, K^T, V, attn out) stay SBUF-resident; the
  f32 baseline spilled Q^T/KV^T to DRAM and re-streamed them.
- V is produced directly in [s, d] layout by swapping matmul operand
  roles (lhsT = x^T tile, rhs = Wv block): no PE transposes.
- KV dedup via one per-batch AllGather (4.2MB bf16, ~106us at the
  observed ~40GB/s) saves ~400 matmul-units/core of duplicated
  projection; the wq weight stream intentionally queues behind the
  gather input so the Q projection stretches across the gather window
  instead of idling the PE (split per-group gathers serialize and lose
  ~22us -- don't).
- Scores for two s_k tiles share one [128,1024] PSUM tile -> 8 wide
  exps/head on ACT instead of 16 (ACT ~= PE time per head otherwise).
- Softmax denominators stay off the PE: e-tiles pair-summed on DVE +
  Pool (late tiles on DVE -- Pool TENSOR_TENSOR is ~2x slower), then 4
  accumulating [4,512] indicator-column matmuls collect all 4 heads'
  row sums into one PSUM tile; ONE batched reciprocal per group (a
  [1,512] reciprocal costs 3.3us of DVE -- per-head recips were 53us),
  broadcast back via indicator-row f32r matmuls.
- Per-head normalization is emitted one head late (psl at next head's
  i==1, recip at i==3, finish at i==5/7) so the in-order PE queue never
  waits on the DVE/Pool tree tail.
- 1/sqrt(128) folded into Wq/bq on host; out-projection reads per-head
  OT tiles so its first matmuls don't serialize on the last head, and
  its bias rows are broadcast into SBUF during the gather window so
  phase 3 runs pure accumulation.
- DMA layouts are g-major end-to-end (wk host layout, Vq/Vs on-chip):
  per-group slices of a g-interleaved layout have 256B lines and run
  at ~1/4 DMA rate (cost a hidden ~14us before the fix).
"""

import numpy as np

E = 2048
S = 2048
P = 128
H = 16
G = 4
SQ = 512          # query rows per core
EB = E // P       # 16 e-blocks (contraction tiles)
NCORES = 8

_NC = None
TRACE = False
LAST_RESULT = None


def _build():
    import concourse.bacc as bacc
    import concourse.mybir as mybir
    import concourse.tile as tile

    f32 = mybir.dt.float32
    f32r = mybir.dt.float32r
    bf16 = mybir.dt.bfloat16
    EXP = mybir.ActivationFunctionType.Exp

    nc = bacc.Bacc("TRN2", target_bir_lowering=False, debug=False,
                   num_devices=NCORES)

    x = nc.declare_dram_parameter("x", [P, EB, 512], bf16, isOutput=False).ap()
    wq = nc.declare_dram_parameter("wq", [H, P, EB, P], bf16, isOutput=False).ap()
    wk = nc.declare_dram_parameter("wk", [P, G, EB, P], bf16, isOutput=False).ap()
    wv = nc.declare_dram_parameter("wv", [P, EB, G * P], bf16, isOutput=False).ap()
    wo = nc.declare_dram_parameter("wo", [P, EB, E], bf16, isOutput=False).ap()
    bq = nc.declare_dram_parameter("bq", [P, H], f32, isOutput=False).ap()
    bk = nc.declare_dram_parameter("bk", [P, G], f32, isOutput=False).ap()
    bvr = nc.declare_dram_parameter("bvr", [1, G * P], f32, isOutput=False).ap()
    bo = nc.declare_dram_parameter("bo", [1, E], f32, isOutput=False).ap()
    oc4d = nc.declare_dram_parameter("oc4d", [P, 4, 4], bf16, isOutput=False).ap()
    selrd = nc.declare_dram_parameter("selrd", [4, 4, P], f32, isOutput=False).ap()
    out = nc.declare_dram_parameter("out", [SQ, E], f32, isOutput=True).ap()

    def r(ap):
        return ap.bitcast(f32r)

    with tile.TileContext(nc) as tc, \
         nc.allow_low_precision(reason="bf16 intermediates; end-to-end rel-err checked"):
        with tc.tile_pool(name="consts", bufs=1) as cp, \
             tc.tile_pool(name="keep", bufs=1) as kp:
            onec = cp.tile([P, 1], bf16, tag="onec")
            nc.vector.memset(onec, 1.0)
            oner0 = cp.tile([1, P], f32, tag="oner0")
            nc.vector.memset(oner0, 1.0)
            oner = cp.tile([1, P], f32, tag="oner")
            nc.vector.tensor_copy(r(oner), oner0)
            # indicator constants for batched softmax-denominator rows
            # (host-prepared): oc4[:, hl] is [128,4] with column hl
            # all-ones (ones-matmul lhsT -> row hl of a [4,512] psum
            # tile); selr[:, hl] is [4,128] with row hl all-ones
            # (broadcast-back lhsT).
            oc4 = cp.tile([P, 4, 4], bf16, tag="oc4")
            selr = cp.tile([4, 4, P], f32, tag="selr")
            bq_s = cp.tile([P, H], f32, tag="bqs")
            bk_s = cp.tile([P, G], f32, tag="bks")
            bv_s = cp.tile([1, G * P], f32, tag="bvs")
            bo_s = cp.tile([1, E], f32, tag="bos")

            qT = kp.tile([P, H, SQ], bf16, tag="qt")    # Q^T per head block
            kT = kp.tile([P, G, EB, P], bf16, tag="kt")  # K^T per group/s-tile
            Vs = kp.tile([P, G, EB, P], bf16, tag="vs")  # V[group, s_tile, d]
            # normalized attn out, one tile per head so phase 3's
            # per-head-block reads don't serialize on the last head
            otl = [kp.tile([P, SQ], bf16, tag=f"ot{h}", name=f"ot{h}")
                   for h in range(H)]

            # ---- Phase 1: projections. Each core holds ONLY its own
            # s-quarter of x^T. It computes K^T/V for that quarter, then
            # an AllGather across the 4 cores of its batch assembles the
            # full-sequence K^T/V while the PE crunches the Q projection
            # (the gather concatenates rank-major = natural s order).
            with tc.tile_pool(name="xp", bufs=1) as xp, \
                 tc.tile_pool(name="wqp", bufs=3) as wqp, \
                 tc.tile_pool(name="wkvp", bufs=1) as wkvp, \
                 tc.tile_pool(name="kvq", bufs=1) as kvq, \
                 tc.tile_pool(name="dramp", bufs=1, space="DRAM") as dramp, \
                 tc.tile_pool(name="ps1", bufs=3, space="PSUM") as ps1, \
                 tc.tile_pool(name="ps1v", bufs=2, space="PSUM") as ps1v:
                x0q = [xp.tile([P, 4, 512], bf16, tag=f"x0q{j}",
                               name=f"x0q{j}") for j in range(4)]
                wk_s = wkvp.tile([P, G, EB, P], bf16, tag="wks")
                nc.sync.dma_start(x0q[0], x[:, 0:4])
                nc.sync.dma_start(wk_s[:, 0], wk[:, 0])
                # small const loads AFTER the first two big ones: their
                # per-DMA queue latency (~5us for six) was gating the
                # first K matmul
                nc.sync.dma_start(bk_s, bk)
                nc.sync.dma_start(r(bv_s), r(bvr))
                nc.sync.dma_start(bq_s, bq)
                nc.sync.dma_start(r(bo_s), r(bo))
                nc.sync.dma_start(oc4, oc4d)
                nc.sync.dma_start(r(selr), r(selrd))
                for j in range(1, 4):
                    nc.sync.dma_start(x0q[j], x[:, 4 * j:4 * (j + 1)])
                for g in range(1, G):
                    nc.sync.dma_start(wk_s[:, g], wk[:, g])
                wv_s = wkvp.tile([P, EB, G * P], bf16, tag="wvs")
                nc.sync.dma_start(wv_s, wv)
                kTq = kvq.tile([P, G, 4, P], bf16, tag="ktq")
                Vq = kvq.tile([P, G, 4, P], bf16, tag="vq")
                kv_in = dramp.tile([2, P, 4, 4, P], bf16, tag="kvin")
                kv_ag = dramp.tile([4, 2, P, 4, 4, P], bf16, tag="kvag")

                # K^T / V for this core's own s-quarter
                for g in range(G):
                    ps = ps1.tile([P, 4, P], f32, tag="psk")
                    for b in range(EB):
                        nc.tensor.matmul(
                            ps, wk_s[:, g, b], x0q[b // 4][:, b % 4],
                            start=(b == 0), stop=(b == EB - 1))
                    nc.vector.tensor_scalar_add(kTq[:, g], ps,
                                                bk_s[:, g:g + 1])
                nc.sync.dma_start(kv_in[0], kTq)
                for t in range(4):
                    ps = ps1v.tile([P, G, P], f32, tag="psv")
                    nc.tensor.matmul(ps, r(oner), r(bv_s),
                                     start=True, stop=False)
                    for b in range(EB):
                        nc.tensor.matmul(
                            ps, x0q[b // 4][:, b % 4, t * P:(t + 1) * P],
                            wv_s[:, b],
                            start=False, stop=(b == EB - 1))
                    nc.scalar.copy(Vq[:, :, t, :], ps)

                nc.sync.dma_start(kv_in[1], Vq)
                nc.gpsimd.collective_compute(
                    "AllGather",
                    mybir.AluOpType.bypass,
                    replica_groups=[[0, 1, 2, 3], [4, 5, 6, 7]],
                    ins=[kv_in.opt()],
                    outs=[kv_ag.opt()],
                )

                # out-proj bias rows broadcast to all partitions now,
                # while the gather runs; phase 3 then skips its 16
                # psum-seed matmuls
                bobt = kp.tile([P, 4, 512], f32, tag="bobt")
                for n in range(4):
                    psb = ps1.tile([P, SQ], f32, tag="ps")
                    nc.tensor.matmul(psb, r(oner),
                                     r(bo_s[:, 512 * n:512 * (n + 1)]),
                                     start=True, stop=True)
                    nc.vector.tensor_copy(bobt[:, n], psb)

                # Q projection: QT[d, s_q] for 16 head blocks. Overlaps
                # the AllGather; the wq stream intentionally queues
                # behind kv_in so Q stretches across the gather window
                # instead of finishing early and idling the PE.
                wqts = [wqp.tile([P, EB, P], bf16, tag="wqm", name="wqm")
                        for m in range(3)]
                for m in range(3):
                    nc.sync.dma_start(wqts[m], wq[m])
                for m in range(H):
                    wqm = wqts[m]
                    if m + 3 < H:
                        w_next = wqp.tile([P, EB, P], bf16, tag="wqm")
                        nc.sync.dma_start(w_next, wq[m + 3])
                        wqts.append(w_next)
                    ps = ps1.tile([P, SQ], f32, tag="ps")
                    for b in range(EB):
                        nc.tensor.matmul(ps, wqm[:, b], x0q[b // 4][:, b % 4],
                                         start=(b == 0), stop=(b == EB - 1))
                    nc.vector.tensor_scalar_add(qT[:, m], ps, bq_s[:, m:m + 1])

                # unpack the gathered K^T / V (natural s order)
                for g in range(G):
                    for q in range(4):
                        nc.sync.dma_start(kT[:, g, 4 * q:4 * (q + 1)],
                                          kv_ag[q, 0, :, g])
                        nc.sync.dma_start(Vs[:, g, 4 * q:4 * (q + 1)],
                                          kv_ag[q, 1, :, g])

            # ---- Phase 2: attention. scores^T for two s_k tiles land in
            # one [128,1024] PSUM tile -> one exp -> two attn@V matmuls.
            # Denominator: DVE+Pool tree-sum of e tiles, one f32r
            # ones-matmul, reciprocal, K=1 broadcast matmul.
            with tc.tile_pool(name="wop", bufs=1) as wop:
                wo_s = wop.tile([P, EB, E], bf16, tag="wos")
                nc.sync.dma_start(wo_s, wo)  # prefetch for phase 3

                with tc.tile_pool(name="exq", bufs=6) as exq, \
                     tc.tile_pool(name="accp", bufs=8) as accp, \
                     tc.tile_pool(name="lsb", bufs=2) as lsb, \
                     tc.tile_pool(name="psfp", bufs=5) as psfp, \
                     tc.tile_pool(name="pscp", bufs=2, space="PSUM") as pscp, \
                     tc.tile_pool(name="psop", bufs=2, space="PSUM") as psop, \
                     tc.tile_pool(name="pslp", bufs=1, space="PSUM") as pslp, \
                     tc.tile_pool(name="psbp", bufs=1, space="PSUM") as psbp:
                    # pipelined per-head state
                    pend = [None]   # (hl, a01, a23, psl4) awaiting psl matmuls
                    recq = []       # [(psl4, li4)] groups awaiting reciprocal
                    finq = []       # [(h, hl, psof, li4)] awaiting plb+mul

                    def emit_psl():
                        # denominator rows: 4 accumulating [4,512] matmuls
                        # straight off the two level-2 tree partials
                        hl_p, a01_p, a23_p, psl4_p = pend[0]
                        for k, rhs in enumerate(
                                (a01_p[:, 0:512], a01_p[:, 512:1024],
                                 a23_p[:, 0:512], a23_p[:, 512:1024])):
                            nc.tensor.matmul(psl4_p, oc4[:, hl_p], rhs,
                                             start=(hl_p == 0 and k == 0),
                                             stop=(hl_p == 3 and k == 3))
                        pend[0] = None

                    def emit_recip():
                        psl4_p, li4, staged = recq.pop(0)
                        nc.vector.reciprocal(r(li4), psl4_p)
                        finq.extend(staged)

                    def emit_fin():
                        h_p, hl_p, psof, li4 = finq.pop(0)
                        plb = psbp.tile([P, SQ], f32, tag="plb")
                        nc.tensor.matmul(plb, r(selr[:, hl_p]), r(li4),
                                         start=True, stop=True)
                        lbs = lsb.tile([P, SQ], f32, tag="lbs")
                        nc.vector.tensor_copy(lbs, plb)
                        nc.gpsimd.tensor_mul(otl[h_p], psof, lbs)

                    for g in range(G):
                        psl4 = pslp.tile([4, SQ], f32, tag="psl4")
                        li4 = lsb.tile([4, SQ], f32, tag="li4")
                        grp_stage = []
                        for hl in range(4):
                            h = 4 * g + hl
                            qh = qT[:, h]
                            pso = psop.tile([P, SQ], f32, tag="pso")
                            exps = [None] * 8

                            def sc(i, g=g, qh=qh, exps=exps):
                                # two s_k tiles -> one [128,1024] psum
                                p = pscp.tile([P, 1024], f32, tag="psc")
                                for u in range(2):
                                    t = 2 * i + u
                                    nc.tensor.matmul(
                                        p[:, 512 * u:512 * (u + 1)],
                                        kT[:, g, t], qh,
                                        start=True, stop=True)
                                e = exq.tile([P, 1024], bf16, tag="ex")
                                nc.scalar.activation(e, p, EXP)
                                exps[i] = e

                            def av(i, g=g, pso=pso, exps=exps):
                                e = exps[i]
                                for u in range(2):
                                    t = 2 * i + u
                                    nc.tensor.matmul(
                                        pso, Vs[:, g, t],
                                        e[:, 512 * u:512 * (u + 1)],
                                        start=(i == 0 and u == 0),
                                        stop=(i == 7 and u == 1))

                            sc(0)
                            sc(1)
                            lvl1 = []
                            for i in range(8):
                                if i + 2 < 8:
                                    sc(i + 2)
                                if i == 1 and pend[0] is not None:
                                    emit_psl()
                                if i == 3 and recq:
                                    emit_recip()
                                if i in (5, 7) and finq:
                                    emit_fin()
                                av(i)
                                if i % 2 == 1:
                                    a = accp.tile([P, 1024], bf16, tag="acc")
                                    # early tiles on the slow engine, late
                                    # tiles on DVE (short tail chain)
                                    eng = nc.gpsimd if i in (1, 3) \
                                        else nc.vector
                                    eng.tensor_add(a, exps[i - 1], exps[i])
                                    lvl1.append(a)
                            a01 = accp.tile([P, 1024], bf16, tag="acc")
                            nc.gpsimd.tensor_add(a01, lvl1[0], lvl1[1])
                            a23 = accp.tile([P, 1024], bf16, tag="acc")
                            nc.vector.tensor_add(a23, lvl1[2], lvl1[3])
                            # spill pso to SBUF (frees PSUM for next head)
                            psof = psfp.tile([P, SQ], f32, tag="psof")
                            nc.vector.tensor_copy(psof, pso)
                            pend[0] = (hl, a01, a23, psl4)
                            grp_stage.append((h, hl, psof, li4))
                        recq.append((psl4, li4, grp_stage))
                    emit_psl()
                    while recq:
                        emit_recip()
                    while finq:
                        emit_fin()

                # ---- Phase 3: output projection. out[s_q, eo] accumulates
                # over 16 head blocks; bias seeded via a K=1 ones matmul.
                with tc.tile_pool(name="obp", bufs=3) as obp, \
                     tc.tile_pool(name="ps3", bufs=2, space="PSUM") as ps3p:
                    for n in range(4):
                        for ms in range(4):
                            ps = ps3p.tile([P, 512], f32, tag="ps")
                            for k in range(EB):
                                nc.tensor.matmul(
                                    ps, otl[k][:, ms * P:(ms + 1) * P],
                                    wo_s[:, k, 512 * n:512 * (n + 1)],
                                    start=(k == 0), stop=(k == EB - 1))
                            ob = obp.tile([P, 512], f32, tag="ob")
                            nc.vector.tensor_add(ob, ps, bobt[:, n])
                            nc.sync.dma_start(
                                out[ms * P:(ms + 1) * P, 512 * n:512 * (n + 1)], ob)

    nc.compile()
    return nc


def _get_nc():
    global _NC
    if _NC is None:
        _NC = _build()
    return _NC


def kernel(x, Wq, bq, Wkv, bkv, Wo, bo):
    from concourse.bass_utils import run_bass_kernel_spmd
    import ml_dtypes
    global LAST_RESULT

    bft = ml_dtypes.bfloat16
    x = np.asarray(x, np.float32)
    Wq = np.asarray(Wq, np.float32)
    bq = np.asarray(bq, np.float32)
    Wkv = np.asarray(Wkv, np.float32)
    bkv = np.asarray(bkv, np.float32)
    Wo = np.asarray(Wo, np.float32)
    bo = np.asarray(bo, np.float32)

    nc = _get_nc()
    sc = 1.0 / np.sqrt(E // H)
    # [m, p, b, d]: lhsT tile for Q M-tile m, e-block b
    wq_h = np.ascontiguousarray(
        (Wq * sc).reshape(EB, P, H, P).transpose(2, 1, 0, 3)).astype(bft)
    # K / V column split of Wkv ([K0 V0 K1 V1 ...] blocks of 128)
    kcols = np.concatenate(
        [np.arange(g * 2 * P, g * 2 * P + P) for g in range(G)])
    vcols = kcols + P
    wk_h = np.ascontiguousarray(
        Wkv[:, kcols].reshape(EB, P, G, P).transpose(1, 2, 0, 3)).astype(bft)
    wv_h = np.ascontiguousarray(
        Wkv[:, vcols].reshape(EB, P, G * P).transpose(1, 0, 2)).astype(bft)
    wo_h = np.ascontiguousarray(
        Wo.reshape(EB, P, E).transpose(1, 0, 2)).astype(bft)
    bq_h = np.ascontiguousarray((bq * sc).reshape(H, P).T)
    bk_h = np.ascontiguousarray(bkv[kcols].reshape(G, P).T)
    bv_h = np.ascontiguousarray(bkv[vcols].reshape(1, G * P))
    bo_h = np.ascontiguousarray(bo.reshape(1, E))
    oc4_h = np.ascontiguousarray(
        np.broadcast_to(np.eye(4, dtype=np.float32), (P, 4, 4))).astype(bft)
    selr_h = np.ascontiguousarray(
        np.repeat(np.eye(4, dtype=np.float32), P, axis=1).reshape(4, 4, P))

    in_maps = []
    for c in range(NCORES):
        b, q = divmod(c, 4)
        # this core's own s-quarter of x^T, [p, eb, 512]
        xt_h = np.ascontiguousarray(
            x[b].T[:, 512 * q:512 * (q + 1)].astype(bft)
            .reshape(EB, P, 512).transpose(1, 0, 2))
        in_maps.append({"x": xt_h, "wq": wq_h, "wk": wk_h, "wv": wv_h,
                        "wo": wo_h, "bq": bq_h, "bk": bk_h, "bvr": bv_h,
                        "bo": bo_h, "oc4d": oc4_h, "selrd": selr_h})

    res = run_bass_kernel_spmd(nc, in_maps, core_ids=list(range(NCORES)),
                               trace=TRACE)
    LAST_RESULT = res

    outf = np.empty((2, S, E), np.float32)
    for c in range(NCORES):
        b, q = divmod(c, 4)
        outf[b, 512 * q:512 * (q + 1), :] = res.results[c]["out"]
    return outf
